# revision 29
# baseline (speedup 1.0000x reference)
"""RBF kernel layer (retrieval_knn): out = exp(-||x - p||^2) for x [131072, 64]
against 512 prototypes, distributed data-parallel over 8 NeuronCores.

Math: exp(-dist2) = exp(2*S) where S[n,m] = cross[n,m] - p_sq[m]/2 - x_sq[n]/2,
computed entirely in two bf16 hi/lo-split GEMMs accumulating in fp32 PSUM:
  mm1: [xh_t; nxsq_h; nxsq_l; 1; 1].T @ [ph; 1; 1; npsq_h; npsq_l]  (K=68)
  mm2: [xh_t; xl_t].T @ [pl; ph]                                    (K=128)
where x = xh + xl, p = ph + pl (bf16 splits; the dropped xl@pl term is
~2^-18), npsq* = bf16 split of -p_sq/2, nxsq* = bf16 split of -x_sq/2.

x arrives as xhl=[xh|xl] [nshard, 128] bf16 row-major; ONE hardware xbar
DMA-transpose per XCHUNK tiles lands [xh_t; xl_t] directly in SBUF (no PE
transpose, no PSUM staging, no DVE transpose copies). The exp has no
per-tile bias, so one ACTIVATE covers OCHUNK tiles' PSUM banks and one DMA
stores OCHUNK tiles. DMA instruction count is minimized because each HWDGE
dma_start costs the issuing engine ~600 ns of descriptor generation.
"""

import numpy as np

# Problem constants (hardcoded per harness contract; kernel.py is self-contained)
N = 131072
D = 64
M = 512
GAMMA = 1.0
NCORES = 8
NSHARD = N // NCORES  # 16384
P = 128
K1 = D + 4  # mm1 contraction: 64 xh rows + 2 xsq rows + 2 ones rows
LHS_SLOTS = 4  # manual rotation slots for A (ones rows initialized once)
XCHUNK = 8  # x tiles per transposed input DMA
OCHUNK = 4  # output tiles per ACTIVATE + output DMA (PSUM 4-bank group)

_cache = {}


def _build_bass(nshard=NSHARD):
    import concourse.mybir as mybir
    import concourse.tile as tile
    from concourse import bacc

    f32 = mybir.dt.float32
    bf16 = mybir.dt.bfloat16
    nt = nshard // P
    assert nt % XCHUNK == 0 and XCHUNK % OCHUNK == 0

    nc = bacc.Bacc(None, target_bir_lowering=False)
    # pre-transposed on host: [p, i*P + j] = [xh|xl] feature p of point i*P+j
    xhl_d = nc.dram_tensor("xhl", [P, nshard], bf16, kind="ExternalInput")
    # rows (-x_sq/2 hi, -x_sq/2 lo, ones, ones) in bf16, [4, i*P+p] layout
    nxsq_d = nc.dram_tensor("nxsq", [4, nt * P], bf16, kind="ExternalInput")
    rhs1_d = nc.dram_tensor("rhs1", [K1, M], bf16, kind="ExternalInput")
    rhs2_d = nc.dram_tensor("rhs2", [2 * D, M], bf16, kind="ExternalInput")
    out_d = nc.dram_tensor("out", [nshard, M], f32, kind="ExternalOutput")

    with tile.TileContext(nc) as tc:
        with (
            tc.tile_pool(name="singles", bufs=1) as singles,
            tc.tile_pool(name="outp", bufs=5) as outp,
            tc.tile_pool(name="ps_o", bufs=2, space="PSUM") as ps_o,
        ):
            rhs1_sb = singles.tile([K1, M], bf16)
            nc.sync.dma_start(rhs1_sb[:], rhs1_d[:])
            rhs2_sb = singles.tile([2 * D, M], bf16)
            nc.sync.dma_start(rhs2_sb[:], rhs2_d[:])
            nxsq_sb = singles.tile([4, nt * P], bf16)
            nc.sync.dma_start(nxsq_sb[:], nxsq_d[:])

            # A slots [68, 128]: rows 0..63 = xh_t, 64..67 =
            # [-x_sq/2 hi; -x_sq/2 lo; 1; 1] (copied per tile from the
            # host-packed nxsq rows; start partition 64 is AP-legal).
            a_slots = []
            for j in range(LHS_SLOTS):
                A_sb = singles.tile([K1, P], bf16, name=f"A{j}")
                a_slots.append(A_sb)

            # x arrives pre-transposed; all 4 MB stays resident in SBUF for
            # the whole kernel. Chunked into XCHUNK-tile copy DMAs (fully
            # contiguous per partition) so compute starts after the first.
            X_all = singles.tile([P, nt * P], bf16)
            for c in range(nt // XCHUNK):
                cs = slice(c * XCHUNK * P, (c + 1) * XCHUNK * P)
                nc.sync.dma_start(X_all[:, cs], xhl_d[:, cs])

            for i in range(nt):
                k = i % OCHUNK
                if k == 0:
                    o_sb = outp.tile([P, OCHUNK, M], f32, tag="o")
                    psum = ps_o.tile([P, OCHUNK, M], f32, tag="psum")

                T = X_all[:, i * P : (i + 1) * P]
                ts = slice(i * P, (i + 1) * P)
                A = a_slots[i % LHS_SLOTS]
                nc.vector.tensor_copy(A[0:D, :], X_all[0:D, ts])
                nc.vector.tensor_copy(A[D:K1, :], nxsq_sb[:, ts])
                nc.tensor.matmul(
                    psum[:, k, :], A[:], rhs1_sb[:], start=True, stop=False
                )
                nc.tensor.matmul(
                    psum[:, k, :], T, rhs2_sb[:], start=False, stop=True
                )

                if k == OCHUNK - 1:
                    # out = exp(2*S) over all OCHUNK PSUM banks at once
                    nc.scalar.activation(
                        o_sb[:],
                        psum[:],
                        mybir.ActivationFunctionType.Exp,
                        bias=0.0,
                        scale=2.0,
                    )
                    i0 = i - (OCHUNK - 1)
                    dest = out_d[i0 * P : (i0 + OCHUNK) * P, :].rearrange(
                        "(t p) m -> p t m", t=OCHUNK
                    )
                    nc.sync.dma_start(dest, o_sb[:])

    nc.finalize()
    return nc


def _get_nc():
    if "nc" not in _cache:
        _cache["nc"] = _build_bass()
    return _cache["nc"]


def _prep_core_arrays(x, prototypes, nshard):
    """Build per-core host arrays (xhl row-major, nxsq, rhs1/rhs2)."""
    import ml_dtypes

    bf = ml_dtypes.bfloat16
    x = np.ascontiguousarray(np.asarray(x, dtype=np.float32))
    prototypes = np.ascontiguousarray(np.asarray(prototypes, dtype=np.float32))

    xh = x.astype(bf)
    xl = (x - xh.astype(np.float32)).astype(bf)
    # [128, N]: rows 0..63 = xh features, 64..127 = xl features
    xhl_t = np.ascontiguousarray(
        np.concatenate([xh, xl], axis=1).T
    )

    nxsq = (-0.5 * (x.astype(np.float64) ** 2).sum(axis=1)).astype(np.float32)
    nxh = nxsq.astype(bf)
    nxl = (nxsq - nxh.astype(np.float32)).astype(bf)

    pt = prototypes.T.astype(np.float32)  # [64, 512]
    ph = pt.astype(bf)
    pl = (pt - ph.astype(np.float32)).astype(bf)

    p_sq = (prototypes.astype(np.float64) ** 2).sum(axis=1)  # [512]
    t = (-0.5 * p_sq).astype(np.float32)
    th = t.astype(bf)
    tl = (t - th.astype(np.float32)).astype(bf)

    ones = np.ones((1, M), dtype=bf)
    # row order matches A: [xh_t rows; nxsq h/l rows; ones rows]
    rhs1 = np.ascontiguousarray(
        np.concatenate([ph, ones, ones, th[None, :], tl[None, :]], axis=0)
    )  # [68, 512] bf16
    rhs2 = np.ascontiguousarray(np.concatenate([pl, ph], axis=0))  # [128, 512]

    ncores = x.shape[0] // nshard
    in_maps = []
    for s in range(ncores):
        sl = slice(s * nshard, (s + 1) * nshard)
        ones_row = np.ones(nshard, dtype=bf)
        nxsq_r = np.ascontiguousarray(
            np.stack([nxh[sl], nxl[sl], ones_row, ones_row], axis=0)
        )
        in_maps.append(
            {
                "xhl": np.ascontiguousarray(xhl_t[:, sl]),
                "nxsq": nxsq_r,
                "rhs1": rhs1,
                "rhs2": rhs2,
            }
        )
    return in_maps


def _prep_inputs(x, prototypes):
    return _prep_core_arrays(x, prototypes, NSHARD)


def _run(inputs, trace=False):
    from concourse.bass_utils import run_bass_kernel_spmd

    in_maps = _prep_inputs(inputs["x"], inputs["prototypes"])
    nc = _get_nc()
    res = run_bass_kernel_spmd(
        nc, in_maps, core_ids=list(range(NCORES)), trace=trace
    )
    out = np.concatenate([r["out"] for r in res.results], axis=0)
    return out, res


def kernel(**inputs) -> np.ndarray:
    out, _ = _run(inputs, trace=False)
    return out
